# revision 23
# baseline (speedup 1.0000x reference)
"""Trainium2 Bass kernel for nn_CrossNet (topk_masking), v2.

Algorithm (per image of R=512 ROIs, C=81 classes):
  L = xaug @ G_aug @ xaug^T            (G_aug = [[s WqWk^T, s Wq bk],[s bq Wk^T, s bq.bk]],
                                        folded on host; xaug = [x, 1])
  E = exp(L), denom_i = sum_j E_ij     (ACT exp with accum)
  e10 per row via chunked max8 (4 chunks of 128 -> 32 cands -> top8 -> zero -> next8)
  TE = E * (E >= e10)                  (bf16 4x-mode DVE pass)
  M[j,c] = x[j,c] * (x[j,c]==rowmax)   (one-hot scatter matrix, pool stt)
  W[i,c] = (x==rowmax) / denom_i       (pool stt)
  TE^T, W^T via DMA xbar transposes    (DRAM round-trip; zero PE/ACT/DVE cost)
  r^T[c,i] = sum_j M[j,c] TE^T[j,i]    (4 wide bf16 matmuls)
  P^T = prior_zdT^T @ W^T              (prior gather incl. 1/denom and li!=lj mask)
  out^T = sigmoid(relu(r^T) * P^T @ Wfc + b)  (sigmoid = 0.5 + 0.5 tanh(z/2))
Output is produced transposed per image ([C, R]); the host un-transposes.

Engine budget per image: ACT = 4 exp + u-copy + relu-copy + tanh; DVE = 19 small
selection ops + 4 mask stt + pt-copy; Pool = M/W build + rowmax + fc-mult +
final affine; PE = 11 matmuls, no transposes.

Sharding: data-parallel over the 128-image batch, 16 images per core, weights
replicated. Inputs/outputs are full tensors; shard/gather happens on host.
"""

import sys
from contextlib import ExitStack

import numpy as np

sys.path.insert(0, "/opt/trn_rl_repo")

import ml_dtypes

import concourse.bass as bass
import concourse.tile as tile
from concourse import mybir
from concourse.bass_utils import run_bass_kernel_spmd

B, R, C, DK, RK = 128, 512, 81, 64, 10
NCORES = 8
IMG_PER_CORE = B // NCORES          # 16
ROWS_PER_CORE = IMG_PER_CORE * R    # 8192
NT = R // 128                       # 4 row-tiles per image
CP = 128                            # padded class dim for DMA transposes
F32 = mybir.dt.float32
F32R = mybir.dt.float32r
BF16 = mybir.dt.bfloat16
AF = mybir.ActivationFunctionType
OP = mybir.AluOpType


def _build_bass():
    nc = bass.Bass()

    x_d = nc.dram_tensor("x", [ROWS_PER_CORE, C], F32, kind="ExternalInput")
    # host-interleaved for 4-image batched xbar transposes:
    # row (g*R + i) within a 4-image group stores image g's row i at
    # DRAM position (i*4 + g) so one [4R, CP] transpose yields
    # out[p, g, i] = xaug[img g, row i, class p].
    xbf_d = nc.dram_tensor("x_bfp", [ROWS_PER_CORE, CP], BF16,
                           kind="ExternalInput")
    g_d = nc.dram_tensor("g_aug", [C + 1, C + 1], BF16, kind="ExternalInput")
    pr_d = nc.dram_tensor("prior_zdT", [C, C], BF16, kind="ExternalInput")
    wfc_d = nc.dram_tensor("wfc_pad", [C + 1, C], BF16, kind="ExternalInput")
    ones_d = nc.dram_tensor("ones_b", [1, R], BF16, kind="ExternalInput")
    out_d = nc.dram_tensor("out", [IMG_PER_CORE, C, R], F32,
                           kind="ExternalOutput")

    # per-image DRAM views
    x_v = x_d.rearrange("(b ic p) c -> b p ic c", b=IMG_PER_CORE, ic=NT, p=128)
    xbf_v = xbf_d.rearrange("(b r) c -> b r c", b=IMG_PER_CORE, r=R)

    with TileKernel(nc) as tk:
        tk.run(x_v, xbf_v, g_d, pr_d, wfc_d, ones_d, out_d)
    import bass_rust
    bass_rust.move_matmul_waits_to_ldweights(nc.m)
    bass_rust.generate_event_semaphores(nc)
    return nc


class TileKernel:
    def __init__(self, nc):
        self.nc = nc
        self.ctx = ExitStack()

    def __enter__(self):
        self.tc = self.ctx.enter_context(tile.TileContext(self.nc))
        return self

    def __exit__(self, *exc):
        return self.ctx.__exit__(*exc)

    def run(self, x_v, xbf_v, g_d, pr_d, wfc_d, ones_d, out_d):
        nc, tc, ctx = self.nc, self.tc, self.ctx

        singles = ctx.enter_context(tc.tile_pool(name="singles", bufs=1))
        p = {}
        p["x"] = ctx.enter_context(tc.tile_pool(name="sb_x", bufs=5))
        p["xt"] = ctx.enter_context(tc.tile_pool(name="sb_xt", bufs=2))
        p["u"] = ctx.enter_context(tc.tile_pool(name="sb_u", bufs=2))
        p["e"] = ctx.enter_context(tc.tile_pool(name="sb_e", bufs=3))
        p["sel"] = ctx.enter_context(tc.tile_pool(name="sb_sel", bufs=2))
        p["small"] = ctx.enter_context(tc.tile_pool(name="sb_small", bufs=4))
        p["te"] = ctx.enter_context(tc.tile_pool(name="sb_te", bufs=3))
        p["mw"] = ctx.enter_context(tc.tile_pool(name="sb_mw", bufs=6))
        p["tet"] = ctx.enter_context(tc.tile_pool(name="sb_tet", bufs=4))
        p["fc"] = ctx.enter_context(tc.tile_pool(name="sb_fc", bufs=3))
        p["out"] = ctx.enter_context(tc.tile_pool(name="sb_out", bufs=3))
        # DRAM staging for the TE+W xbar transpose
        p["dte"] = ctx.enter_context(
            tc.tile_pool(name="dr_te", bufs=4, space="DRAM"))
        # PSUM pools: u(1) + l(2x2) + rp(2) + o(1) = 8 banks
        p["psu"] = ctx.enter_context(
            tc.tile_pool(name="ps_u", bufs=1, space="PSUM"))
        p["psl"] = ctx.enter_context(
            tc.tile_pool(name="ps_l", bufs=2, space="PSUM"))
        p["psrp"] = ctx.enter_context(
            tc.tile_pool(name="ps_rp", bufs=1, space="PSUM"))
        p["pso"] = ctx.enter_context(
            tc.tile_pool(name="ps_o", bufs=1, space="PSUM"))
        self.p = p

        # constants
        self.g_sb = singles.tile([C + 1, C + 1], BF16, name="g_sb")
        nc.sync.dma_start(out=self.g_sb, in_=g_d[:])
        self.pr_sb = singles.tile([C, C], BF16, name="pr_sb")
        nc.sync.dma_start(out=self.pr_sb, in_=pr_d[:])
        self.wfc_sb = singles.tile([C + 1, C], BF16, name="wfc_sb")
        nc.sync.dma_start(out=self.wfc_sb, in_=wfc_d[:])

        # static fc-input tiles with the ones-row (bias via K=82) pre-loaded
        self.fc_static = []
        for i in range(3):
            t = singles.tile([C + 1, R], BF16, name=f"fc_st{i}")
            nc.sync.dma_start(out=t[C:C + 1, :], in_=ones_d[:])
            self.fc_static.append(t)

        state = [dict() for _ in range(IMG_PER_CORE)]
        self.load_x(0, x_v, xbf_v, state[0])
        self.load_x(1, x_v, xbf_v, state[1])
        self.s1_logits(0, state[0])
        for k in range(IMG_PER_CORE + 5):
            if k + 2 < IMG_PER_CORE:
                self.load_x(k + 2, x_v, xbf_v, state[k + 2])
            if k + 1 < IMG_PER_CORE:
                self.s1_logits(k + 1, state[k + 1])
            if k < IMG_PER_CORE:
                self.s2_select(k, state[k])
            if 0 <= k - 1 < IMG_PER_CORE:
                self.s3_transpose(k - 1, state[k - 1])
            if 0 <= k - 3 < IMG_PER_CORE:
                self.s4_scatter_fc(k - 3, state[k - 3], out_d)
                state[k - 3] = None

    def load_x(self, b, x_v, xbf_v, st):
        nc, p = self.nc, self.p
        x_t = p["x"].tile([128, NT, C], F32, name=f"x_{b}", tag="x")
        nc.sync.dma_start(out=x_t, in_=x_v[b])
        st["x"] = x_t
        # x^T (rows 0..80 = classes, 81 = ones, rest 0)
        xt_t = p["xt"].tile([CP, 1, R], BF16, name=f"xt_{b}", tag="xt")
        nc.sync.dma_start_transpose(out=xt_t, in_=xbf_v[b])
        st["xt_tile"] = xt_t
        st["xt_g"] = 0

    def s1_logits(self, b, st):
        nc, p = self.nc, self.p
        xt, g = st["xt_tile"], st["xt_g"]

        # u^T[c',i] = sum_c G_aug[c,c'] xaug^T[c,i]   [82, 512]
        u_ps = p["psu"].tile([C + 1, R], F32, name=f"ups_{b}", tag="u")
        nc.tensor.matmul(out=u_ps, lhsT=self.g_sb, rhs=xt[0:C + 1, g, :])
        u_sb = p["u"].tile([C + 1, R], BF16, name=f"u_{b}", tag="u")
        nc.scalar.activation(out=u_sb, in_=u_ps, func=AF.Copy)

        # logits tiles + exp (+ per-tile denom)
        denom4 = p["small"].tile([128, NT], F32, name=f"den_{b}", tag="den")
        e_t = p["e"].tile([128, NT, R], BF16, name=f"e_{b}", tag="e")
        for h in range(2):
            l_ps = p["psl"].tile([128, 2, R], F32, name=f"l_{b}_{h}", tag="l")
            for j in range(2):
                ic = 2 * h + j
                nc.tensor.matmul(
                    out=l_ps[:, j, :],
                    lhsT=u_sb[:, ic * 128:(ic + 1) * 128],
                    rhs=xt[0:C + 1, g, :],
                )
            for j in range(2):
                ic = 2 * h + j
                nc.scalar.activation(
                    out=e_t[:, ic, :], in_=l_ps[:, j, :], func=AF.Exp,
                    accum_out=denom4[:, ic:ic + 1],
                )
        st["e"] = e_t
        st["denom"] = denom4

    def s2_select(self, b, st):
        nc, p = self.nc, self.p
        x_t, e_t, denom4 = st["x"], st["e"], st["denom"]

        recip4 = p["small"].tile([128, NT], F32, name=f"rec_{b}", tag="rec")
        nc.vector.reciprocal(out=recip4, in_=denom4)
        m4 = p["small"].tile([128, NT], F32, name=f"m4_{b}", tag="m4")
        nc.vector.tensor_reduce(
            out=m4, in_=x_t, axis=mybir.AxisListType.X, op=OP.max,
        )

        # top-10 threshold per row: top8 of each 128-chunk -> 32 cands
        cand = p["sel"].tile([128, NT, 16], BF16, name=f"cand_{b}", tag="cand")
        top8 = p["sel"].tile([128, NT, 8], BF16, name=f"top8_{b}", tag="top8")
        candz = p["sel"].tile([128, NT, 16], BF16, name=f"candz_{b}",
                              tag="candz")
        next8 = p["sel"].tile([128, NT, 8], BF16, name=f"next8_{b}",
                              tag="next8")
        # TE and W share one staging tile so a single xbar transpose
        # produces TE^T (chunks 0..3) and W^T (chunk 4).
        tew = p["te"].tile([128, NT, R + CP], BF16, name=f"tew_{b}", tag="te")
        for ic in range(NT):
            for kc in range(2):
                nc.vector.max(
                    out=cand[:, ic, kc * 8:(kc + 1) * 8],
                    in_=e_t[:, ic, kc * 256:(kc + 1) * 256],
                )
            nc.vector.max(out=top8[:, ic, :], in_=cand[:, ic, :])
            nc.vector.match_replace(
                out=candz[:, ic, :], in_to_replace=top8[:, ic, :],
                in_values=cand[:, ic, :], imm_value=0.0,
            )
            nc.vector.max(out=next8[:, ic, :], in_=candz[:, ic, :])
            # TE = (E >= e10) * E  (single 4x-mode DVE pass)
            nc.vector.scalar_tensor_tensor(
                out=tew[:, ic, 0:R], in0=e_t[:, ic, :],
                scalar=next8[:, ic, 1:2], in1=e_t[:, ic, :],
                op0=OP.is_ge, op1=OP.mult,
            )
        st["e"] = None

        # eqm = (x == rowmax); M = eqm*x ; W = eqm*recip  (mults on pool)
        eqm = p["mw"].tile([128, NT, C], BF16, name=f"eqm_{b}", tag="eqm")
        nc.vector.tensor_tensor(
            out=eqm, in0=x_t, in1=m4.to_broadcast([128, NT, C]),
            op=OP.is_equal,
        )
        m_sb = p["mw"].tile([128, NT, C], BF16, name=f"m_{b}", tag="mm")
        nc.gpsimd.tensor_tensor(out=m_sb, in0=eqm, in1=x_t, op=OP.mult)
        nc.gpsimd.tensor_tensor(
            out=tew[:, :, R:R + C], in0=eqm,
            in1=recip4.to_broadcast([128, NT, C]), op=OP.mult,
        )
        st["m"] = m_sb

        # stage TE|W in DRAM for the xbar transpose
        te_d = p["dte"].tile([R, R + CP], BF16, name=f"ted_{b}", tag="ted")
        nc.sync.dma_start(
            out=te_d.rearrange("(ic p) j -> p ic j", p=128), in_=tew)
        st["te_d"] = te_d

    def s3_transpose(self, b, st):
        nc, p = self.nc, self.p
        # tw[:, jc, i] = TE[i, jc*128+p] for jc<4; tw[:, 4, i] = W[i, p]
        tw = p["tet"].tile([128, NT + 1, R], BF16, name=f"tw_{b}", tag="tet")
        nc.sync.dma_start_transpose(out=tw, in_=st["te_d"][:, :])
        st["tw"] = tw
        st["te_d"] = None

    def s4_scatter_fc(self, b, st, out_d):
        nc, p = self.nc, self.p

        # r^T[c,i] += M[jc]^T @ TE^T[jc]; P^T = prior_zdT^T @ W^T in the
        # adjacent PSUM bank so one ACT relu covers both (P >= 0 always).
        rp_ps = p["psrp"].tile([C, 2, R], F32, name=f"rpps_{b}", tag="rp")
        for jc in range(NT):
            nc.tensor.matmul(
                out=rp_ps[:, 0, :], lhsT=st["m"][:, jc, :],
                rhs=st["tw"][:, jc, :],
                start=(jc == 0), stop=(jc == NT - 1),
            )
        nc.tensor.matmul(out=rp_ps[:, 1, :], lhsT=self.pr_sb,
                         rhs=st["tw"][0:C, NT, :])
        rp = p["fc"].tile([C, 2, R], BF16, name=f"rp_{b}", tag="rp")
        nc.scalar.activation(out=rp, in_=rp_ps, func=AF.Relu)

        # fc_in = relu(r^T) * P^T  into the static padded tile (ones-row at C)
        fc_in = self.fc_static[b % 3]
        nc.gpsimd.tensor_tensor(
            out=fc_in[0:C, :], in0=rp[:, 0, :], in1=rp[:, 1, :], op=OP.mult,
        )

        # out^T = Wfc^T @ fc_in   [81, 512] (K=82 folds bias)
        o_ps = p["pso"].tile([C, R], F32, name=f"ops_{b}", tag="o")
        nc.tensor.matmul(out=o_ps, lhsT=self.wfc_sb, rhs=fc_in)

        # sigmoid via tanh: out = 0.5 + 0.5*tanh(0.5*logits)
        sig = p["out"].tile([C, R], F32, name=f"sig_{b}", tag="sig")
        nc.scalar.activation(out=sig, in_=o_ps, func=AF.Tanh, scale=0.5)
        o_t = p["out"].tile([C, R], F32, name=f"o_{b}", tag="o")
        nc.scalar.activation(out=o_t, in_=sig, func=AF.Copy, scale=0.5,
                             bias=0.5)
        nc.sync.dma_start(out=out_d[b], in_=o_t)


def _install_ntff_hook():
    """Provide antenv.axon_hooks if the image lacks it (profiling only)."""
    import types
    try:
        from antenv.axon_hooks import get_axon_ntff_profile_hook  # noqa: F401
        return
    except ImportError:
        pass
    try:
        from trn_agent_boot.trn_boot import _ntff_profile_via_ctypes
        hook = _ntff_profile_via_ctypes("/opt/axon/libaxon_pjrt.so")
    except Exception:
        hook = None
    mod = types.ModuleType("antenv.axon_hooks")
    mod.get_axon_ntff_profile_hook = lambda: hook
    mod.set_axon_ntff_profile_hook = lambda h: None
    sys.modules["antenv.axon_hooks"] = mod


_NC_CACHE = None


def _get_nc():
    global _NC_CACHE
    if _NC_CACHE is None:
        _NC_CACHE = _build_bass()
    return _NC_CACHE


def kernel(x, Wq, bq, Wk, bk, Wfc, bfc, prior_rel, _trace=False):
    x = np.ascontiguousarray(np.asarray(x, np.float32))
    Wq = np.asarray(Wq, np.float32); bq = np.asarray(bq, np.float32)
    Wk = np.asarray(Wk, np.float32); bk = np.asarray(bk, np.float32)
    Wfc = np.asarray(Wfc, np.float32); bfc = np.asarray(bfc, np.float32)
    prior = np.asarray(prior_rel, np.float32)

    s = np.float32(1.0 / np.sqrt(np.float32(DK)))
    g_aug = np.zeros((C + 1, C + 1), np.float32)
    g_aug[:C, :C] = s * (Wq @ Wk.T)
    g_aug[:C, C] = s * (Wq @ bk)
    g_aug[C, :C] = s * (Wk @ bq)
    g_aug[C, C] = s * float(bq @ bk)
    g_aug = g_aug.astype(ml_dtypes.bfloat16)

    x_bfp = np.zeros((B * R, CP), ml_dtypes.bfloat16)
    x_bfp[:, :C] = x.astype(ml_dtypes.bfloat16)
    x_bfp[:, C] = 1.0

    prior_zd = prior.copy()
    np.fill_diagonal(prior_zd, 0.0)
    prior_zdT = np.ascontiguousarray(prior_zd.T).astype(ml_dtypes.bfloat16)
    wfc_pad = np.vstack([Wfc, bfc[None, :]]).astype(ml_dtypes.bfloat16)

    if _trace:
        sys.path.insert(0, "/root/.axon_site")
        _install_ntff_hook()
    nc = _get_nc()
    in_maps = []
    for c in range(NCORES):
        in_maps.append({
            "x": x[c * ROWS_PER_CORE:(c + 1) * ROWS_PER_CORE],
            "x_bfp": x_bfp[c * ROWS_PER_CORE:(c + 1) * ROWS_PER_CORE],
            "g_aug": g_aug,
            "prior_zdT": prior_zdT,
            "wfc_pad": wfc_pad,
            "ones_b": np.ones((1, R), ml_dtypes.bfloat16),
        })
    res = run_bass_kernel_spmd(nc, in_maps, list(range(NCORES)), trace=_trace)
    # per-core out is [IMG, C, R]; un-transpose to [IMG*R, C]
    out = np.concatenate(
        [np.asarray(r["out"]).transpose(0, 2, 1).reshape(-1, C)
         for r in res.results], axis=0).astype(np.float32)
    if _trace:
        return out, res
    return out


if __name__ == "__main__":
    rng = np.random.default_rng(0)
    inputs = {
        "x": rng.standard_normal((B * R, C), dtype=np.float32),
        "Wq": rng.standard_normal((C, DK), dtype=np.float32) / 9.0,
        "bq": np.zeros(DK, np.float32),
        "Wk": rng.standard_normal((C, DK), dtype=np.float32) / 9.0,
        "bk": np.zeros(DK, np.float32),
        "Wfc": rng.standard_normal((C, C), dtype=np.float32) / 9.0,
        "bfc": np.zeros(C, np.float32),
        "prior_rel": rng.random((C, C), dtype=np.float32),
    }
    out = kernel(**inputs)
    print("out", out.shape, out.dtype, float(out.mean()))


# revision 24
# speedup vs baseline: 1.0291x; 1.0291x over previous
"""Trainium2 Bass kernel for nn_CrossNet (topk_masking), v2.

Algorithm (per image of R=512 ROIs, C=81 classes):
  L = xaug @ G_aug @ xaug^T            (G_aug = [[s WqWk^T, s Wq bk],[s bq Wk^T, s bq.bk]],
                                        folded on host; xaug = [x, 1])
  E = exp(L), denom_i = sum_j E_ij     (ACT exp with accum)
  e10 per row via chunked max8 (4 chunks of 128 -> 32 cands -> top8 -> zero -> next8)
  TE = E * (E >= e10)                  (bf16 4x-mode DVE pass)
  M[j,c] = x[j,c] * (x[j,c]==rowmax)   (one-hot scatter matrix, pool stt)
  W[i,c] = (x==rowmax) / denom_i       (pool stt)
  TE^T, W^T via DMA xbar transposes    (DRAM round-trip; zero PE/ACT/DVE cost)
  r^T[c,i] = sum_j M[j,c] TE^T[j,i]    (4 wide bf16 matmuls)
  P^T = prior_zdT^T @ W^T              (prior gather incl. 1/denom and li!=lj mask)
  out^T = sigmoid(relu(r^T) * P^T @ Wfc + b)  (sigmoid = 0.5 + 0.5 tanh(z/2))
Output is produced transposed per image ([C, R]); the host un-transposes.

Engine budget per image: ACT = 4 exp + u-copy + relu-copy + tanh; DVE = 19 small
selection ops + 4 mask stt + pt-copy; Pool = M/W build + rowmax + fc-mult +
final affine; PE = 11 matmuls, no transposes.

Sharding: data-parallel over the 128-image batch, 16 images per core, weights
replicated. Inputs/outputs are full tensors; shard/gather happens on host.
"""

import sys
from contextlib import ExitStack

import numpy as np

sys.path.insert(0, "/opt/trn_rl_repo")

import ml_dtypes

import concourse.bass as bass
import concourse.tile as tile
from concourse import mybir
from concourse.bass_utils import run_bass_kernel_spmd

B, R, C, DK, RK = 128, 512, 81, 64, 10
NCORES = 8
IMG_PER_CORE = B // NCORES          # 16
ROWS_PER_CORE = IMG_PER_CORE * R    # 8192
NT = R // 128                       # 4 row-tiles per image
CP = 128                            # padded class dim for DMA transposes
F32 = mybir.dt.float32
F32R = mybir.dt.float32r
BF16 = mybir.dt.bfloat16
AF = mybir.ActivationFunctionType
OP = mybir.AluOpType


def _build_bass():
    nc = bass.Bass()

    x_d = nc.dram_tensor("x", [ROWS_PER_CORE, C], F32, kind="ExternalInput")
    # host-interleaved for 4-image batched xbar transposes:
    # row (g*R + i) within a 4-image group stores image g's row i at
    # DRAM position (i*4 + g) so one [4R, CP] transpose yields
    # out[p, g, i] = xaug[img g, row i, class p].
    xbf_d = nc.dram_tensor("x_bfp", [ROWS_PER_CORE, CP], BF16,
                           kind="ExternalInput")
    g_d = nc.dram_tensor("g_aug", [C + 1, C + 1], BF16, kind="ExternalInput")
    pr_d = nc.dram_tensor("prior_zdT", [C, C], BF16, kind="ExternalInput")
    wfc_d = nc.dram_tensor("wfc_pad", [C + 1, C], BF16, kind="ExternalInput")
    ones_d = nc.dram_tensor("ones_b", [1, R], BF16, kind="ExternalInput")
    out_d = nc.dram_tensor("out", [IMG_PER_CORE, C, R], F32,
                           kind="ExternalOutput")

    # per-image DRAM views
    x_v = x_d.rearrange("(b ic p) c -> b p ic c", b=IMG_PER_CORE, ic=NT, p=128)
    xbf_v = xbf_d.rearrange("(b r) c -> b r c", b=IMG_PER_CORE, r=R)

    with TileKernel(nc) as tk:
        tk.run(x_v, xbf_v, g_d, pr_d, wfc_d, ones_d, out_d)
    import bass_rust
    bass_rust.move_matmul_waits_to_ldweights(nc.m)
    bass_rust.generate_event_semaphores(nc)
    return nc


class TileKernel:
    def __init__(self, nc):
        self.nc = nc
        self.ctx = ExitStack()

    def __enter__(self):
        self.tc = self.ctx.enter_context(tile.TileContext(self.nc))
        return self

    def __exit__(self, *exc):
        return self.ctx.__exit__(*exc)

    def run(self, x_v, xbf_v, g_d, pr_d, wfc_d, ones_d, out_d):
        nc, tc, ctx = self.nc, self.tc, self.ctx

        singles = ctx.enter_context(tc.tile_pool(name="singles", bufs=1))
        p = {}
        p["x"] = ctx.enter_context(tc.tile_pool(name="sb_x", bufs=5))
        p["xt"] = ctx.enter_context(tc.tile_pool(name="sb_xt", bufs=2))
        p["u"] = ctx.enter_context(tc.tile_pool(name="sb_u", bufs=2))
        p["e"] = ctx.enter_context(tc.tile_pool(name="sb_e", bufs=3))
        p["sel"] = ctx.enter_context(tc.tile_pool(name="sb_sel", bufs=2))
        p["small"] = ctx.enter_context(tc.tile_pool(name="sb_small", bufs=4))
        p["te"] = ctx.enter_context(tc.tile_pool(name="sb_te", bufs=3))
        p["mw"] = ctx.enter_context(tc.tile_pool(name="sb_mw", bufs=6))
        p["tet"] = ctx.enter_context(tc.tile_pool(name="sb_tet", bufs=4))
        p["fc"] = ctx.enter_context(tc.tile_pool(name="sb_fc", bufs=3))
        p["out"] = ctx.enter_context(tc.tile_pool(name="sb_out", bufs=3))
        # DRAM staging for the TE+W xbar transpose
        p["dte"] = ctx.enter_context(
            tc.tile_pool(name="dr_te", bufs=4, space="DRAM"))
        # PSUM pools: u(1) + l(2x2) + rp(2) + o(1) = 8 banks
        p["psu"] = ctx.enter_context(
            tc.tile_pool(name="ps_u", bufs=1, space="PSUM"))
        p["psl"] = ctx.enter_context(
            tc.tile_pool(name="ps_l", bufs=1, space="PSUM"))
        p["psrp"] = ctx.enter_context(
            tc.tile_pool(name="ps_rp", bufs=2, space="PSUM"))
        p["pso"] = ctx.enter_context(
            tc.tile_pool(name="ps_o", bufs=1, space="PSUM"))
        self.p = p

        # constants
        self.g_sb = singles.tile([C + 1, C + 1], BF16, name="g_sb")
        nc.sync.dma_start(out=self.g_sb, in_=g_d[:])
        self.pr_sb = singles.tile([C, C], BF16, name="pr_sb")
        nc.sync.dma_start(out=self.pr_sb, in_=pr_d[:])
        self.wfc_sb = singles.tile([C + 1, C], BF16, name="wfc_sb")
        nc.sync.dma_start(out=self.wfc_sb, in_=wfc_d[:])

        # static fc-input tiles with the ones-row (bias via K=82) pre-loaded
        self.fc_static = []
        for i in range(3):
            t = singles.tile([C + 1, R], BF16, name=f"fc_st{i}")
            nc.sync.dma_start(out=t[C:C + 1, :], in_=ones_d[:])
            self.fc_static.append(t)

        state = [dict() for _ in range(IMG_PER_CORE)]
        self.load_x(0, x_v, xbf_v, state[0])
        self.load_x(1, x_v, xbf_v, state[1])
        self.s1_logits(0, state[0])
        for k in range(IMG_PER_CORE + 5):
            if 0 <= k - 3 < IMG_PER_CORE:
                self.s4_scatter_fc(k - 3, state[k - 3], out_d)
            if k + 2 < IMG_PER_CORE:
                self.load_x(k + 2, x_v, xbf_v, state[k + 2])
            if k + 1 < IMG_PER_CORE:
                self.s1_logits(k + 1, state[k + 1])
            if k < IMG_PER_CORE:
                self.s2_select(k, state[k])
            if 0 <= k - 1 < IMG_PER_CORE:
                self.s3_transpose(k - 1, state[k - 1])
            if 0 <= k - 3 < IMG_PER_CORE:
                state[k - 3] = None

    def load_x(self, b, x_v, xbf_v, st):
        nc, p = self.nc, self.p
        x_t = p["x"].tile([128, NT, C], F32, name=f"x_{b}", tag="x")
        nc.gpsimd.dma_start(out=x_t, in_=x_v[b])
        st["x"] = x_t
        # x^T (rows 0..80 = classes, 81 = ones, rest 0)
        xt_t = p["xt"].tile([CP, 1, R], BF16, name=f"xt_{b}", tag="xt")
        nc.sync.dma_start_transpose(out=xt_t, in_=xbf_v[b])
        st["xt_tile"] = xt_t
        st["xt_g"] = 0

    def s1_logits(self, b, st):
        nc, p = self.nc, self.p
        xt, g = st["xt_tile"], st["xt_g"]

        # u^T[c',i] = sum_c G_aug[c,c'] xaug^T[c,i]   [82, 512]
        u_ps = p["psu"].tile([C + 1, R], F32, name=f"ups_{b}", tag="u")
        nc.tensor.matmul(out=u_ps, lhsT=self.g_sb, rhs=xt[0:C + 1, g, :])
        u_sb = p["u"].tile([C + 1, R], BF16, name=f"u_{b}", tag="u")
        nc.scalar.activation(out=u_sb, in_=u_ps, func=AF.Copy)

        # logits tiles + exp (+ per-tile denom)
        denom4 = p["small"].tile([128, NT], F32, name=f"den_{b}", tag="den")
        e_t = p["e"].tile([128, NT, R], BF16, name=f"e_{b}", tag="e")
        for h in range(2):
            l_ps = p["psl"].tile([128, 2, R], F32, name=f"l_{b}_{h}", tag="l")
            for j in range(2):
                ic = 2 * h + j
                nc.tensor.matmul(
                    out=l_ps[:, j, :],
                    lhsT=u_sb[:, ic * 128:(ic + 1) * 128],
                    rhs=xt[0:C + 1, g, :],
                )
            for j in range(2):
                ic = 2 * h + j
                nc.scalar.activation(
                    out=e_t[:, ic, :], in_=l_ps[:, j, :], func=AF.Exp,
                    accum_out=denom4[:, ic:ic + 1],
                )
        st["e"] = e_t
        st["denom"] = denom4

    def s2_select(self, b, st):
        nc, p = self.nc, self.p
        x_t, e_t, denom4 = st["x"], st["e"], st["denom"]

        recip4 = p["small"].tile([128, NT], F32, name=f"rec_{b}", tag="rec")
        nc.vector.reciprocal(out=recip4, in_=denom4)
        m4 = p["small"].tile([128, NT], F32, name=f"m4_{b}", tag="m4")
        nc.vector.tensor_reduce(
            out=m4, in_=x_t, axis=mybir.AxisListType.X, op=OP.max,
        )

        # top-10 threshold per row: top8 of each 128-chunk -> 32 cands
        cand = p["sel"].tile([128, NT, 16], BF16, name=f"cand_{b}", tag="cand")
        top8 = p["sel"].tile([128, NT, 8], BF16, name=f"top8_{b}", tag="top8")
        candz = p["sel"].tile([128, NT, 16], BF16, name=f"candz_{b}",
                              tag="candz")
        next8 = p["sel"].tile([128, NT, 8], BF16, name=f"next8_{b}",
                              tag="next8")
        # TE and W share one staging tile so a single xbar transpose
        # produces TE^T (chunks 0..3) and W^T (chunk 4).
        tew = p["te"].tile([128, NT, R + CP], BF16, name=f"tew_{b}", tag="te")
        for ic in range(NT):
            for kc in range(2):
                nc.vector.max(
                    out=cand[:, ic, kc * 8:(kc + 1) * 8],
                    in_=e_t[:, ic, kc * 256:(kc + 1) * 256],
                )
            nc.vector.max(out=top8[:, ic, :], in_=cand[:, ic, :])
            nc.vector.match_replace(
                out=candz[:, ic, :], in_to_replace=top8[:, ic, :],
                in_values=cand[:, ic, :], imm_value=0.0,
            )
            nc.vector.max(out=next8[:, ic, :], in_=candz[:, ic, :])
            # TE = (E >= e10) * E  (single 4x-mode DVE pass)
            nc.vector.scalar_tensor_tensor(
                out=tew[:, ic, 0:R], in0=e_t[:, ic, :],
                scalar=next8[:, ic, 1:2], in1=e_t[:, ic, :],
                op0=OP.is_ge, op1=OP.mult,
            )
        st["e"] = None

        # eqm = (x == rowmax); M = eqm*x ; W = eqm*recip  (mults on pool)
        eqm = p["mw"].tile([128, NT, C], BF16, name=f"eqm_{b}", tag="eqm")
        nc.vector.tensor_tensor(
            out=eqm, in0=x_t, in1=m4.to_broadcast([128, NT, C]),
            op=OP.is_equal,
        )
        m_sb = p["mw"].tile([128, NT, C], BF16, name=f"m_{b}", tag="mm")
        nc.gpsimd.tensor_tensor(out=m_sb, in0=eqm, in1=x_t, op=OP.mult)
        nc.gpsimd.tensor_tensor(
            out=tew[:, :, R:R + C], in0=eqm,
            in1=recip4.to_broadcast([128, NT, C]), op=OP.mult,
        )
        st["m"] = m_sb

        # stage TE|W in DRAM for the xbar transpose
        te_d = p["dte"].tile([R, R + CP], BF16, name=f"ted_{b}", tag="ted")
        nc.sync.dma_start(
            out=te_d.rearrange("(ic p) j -> p ic j", p=128), in_=tew)
        st["te_d"] = te_d

    def s3_transpose(self, b, st):
        nc, p = self.nc, self.p
        # tw[:, jc, i] = TE[i, jc*128+p] for jc<4; tw[:, 4, i] = W[i, p]
        tw = p["tet"].tile([128, NT + 1, R], BF16, name=f"tw_{b}", tag="tet")
        nc.sync.dma_start_transpose(out=tw, in_=st["te_d"][:, :])
        st["tw"] = tw
        st["te_d"] = None

    def s4_scatter_fc(self, b, st, out_d):
        nc, p = self.nc, self.p

        # r^T[c,i] += M[jc]^T @ TE^T[jc]; P^T = prior_zdT^T @ W^T in the
        # adjacent PSUM bank so one ACT relu covers both (P >= 0 always).
        rp_ps = p["psrp"].tile([C, 2, R], F32, name=f"rpps_{b}", tag="rp")
        for jc in range(NT):
            nc.tensor.matmul(
                out=rp_ps[:, 0, :], lhsT=st["m"][:, jc, :],
                rhs=st["tw"][:, jc, :],
                start=(jc == 0), stop=(jc == NT - 1),
            )
        nc.tensor.matmul(out=rp_ps[:, 1, :], lhsT=self.pr_sb,
                         rhs=st["tw"][0:C, NT, :])
        rp = p["fc"].tile([C, 2, R], BF16, name=f"rp_{b}", tag="rp")
        nc.scalar.activation(out=rp, in_=rp_ps, func=AF.Relu)

        # fc_in = relu(r^T) * P^T  into the static padded tile (ones-row at C)
        fc_in = self.fc_static[b % 3]
        nc.gpsimd.tensor_tensor(
            out=fc_in[0:C, :], in0=rp[:, 0, :], in1=rp[:, 1, :], op=OP.mult,
        )

        # out^T = Wfc^T @ fc_in   [81, 512] (K=82 folds bias)
        o_ps = p["pso"].tile([C, R], F32, name=f"ops_{b}", tag="o")
        nc.tensor.matmul(out=o_ps, lhsT=self.wfc_sb, rhs=fc_in)

        # sigmoid via tanh: out = 0.5 + 0.5*tanh(0.5*logits)
        sig = p["out"].tile([C, R], F32, name=f"sig_{b}", tag="sig")
        nc.scalar.activation(out=sig, in_=o_ps, func=AF.Tanh, scale=0.5)
        o_t = p["out"].tile([C, R], F32, name=f"o_{b}", tag="o")
        nc.scalar.activation(out=o_t, in_=sig, func=AF.Copy, scale=0.5,
                             bias=0.5)
        nc.gpsimd.dma_start(out=out_d[b], in_=o_t)


def _install_ntff_hook():
    """Provide antenv.axon_hooks if the image lacks it (profiling only)."""
    import types
    try:
        from antenv.axon_hooks import get_axon_ntff_profile_hook  # noqa: F401
        return
    except ImportError:
        pass
    try:
        from trn_agent_boot.trn_boot import _ntff_profile_via_ctypes
        hook = _ntff_profile_via_ctypes("/opt/axon/libaxon_pjrt.so")
    except Exception:
        hook = None
    mod = types.ModuleType("antenv.axon_hooks")
    mod.get_axon_ntff_profile_hook = lambda: hook
    mod.set_axon_ntff_profile_hook = lambda h: None
    sys.modules["antenv.axon_hooks"] = mod


_NC_CACHE = None


def _get_nc():
    global _NC_CACHE
    if _NC_CACHE is None:
        _NC_CACHE = _build_bass()
    return _NC_CACHE


def kernel(x, Wq, bq, Wk, bk, Wfc, bfc, prior_rel, _trace=False):
    x = np.ascontiguousarray(np.asarray(x, np.float32))
    Wq = np.asarray(Wq, np.float32); bq = np.asarray(bq, np.float32)
    Wk = np.asarray(Wk, np.float32); bk = np.asarray(bk, np.float32)
    Wfc = np.asarray(Wfc, np.float32); bfc = np.asarray(bfc, np.float32)
    prior = np.asarray(prior_rel, np.float32)

    s = np.float32(1.0 / np.sqrt(np.float32(DK)))
    g_aug = np.zeros((C + 1, C + 1), np.float32)
    g_aug[:C, :C] = s * (Wq @ Wk.T)
    g_aug[:C, C] = s * (Wq @ bk)
    g_aug[C, :C] = s * (Wk @ bq)
    g_aug[C, C] = s * float(bq @ bk)
    g_aug = g_aug.astype(ml_dtypes.bfloat16)

    x_bfp = np.zeros((B * R, CP), ml_dtypes.bfloat16)
    x_bfp[:, :C] = x.astype(ml_dtypes.bfloat16)
    x_bfp[:, C] = 1.0

    prior_zd = prior.copy()
    np.fill_diagonal(prior_zd, 0.0)
    prior_zdT = np.ascontiguousarray(prior_zd.T).astype(ml_dtypes.bfloat16)
    wfc_pad = np.vstack([Wfc, bfc[None, :]]).astype(ml_dtypes.bfloat16)

    if _trace:
        sys.path.insert(0, "/root/.axon_site")
        _install_ntff_hook()
    nc = _get_nc()
    in_maps = []
    for c in range(NCORES):
        in_maps.append({
            "x": x[c * ROWS_PER_CORE:(c + 1) * ROWS_PER_CORE],
            "x_bfp": x_bfp[c * ROWS_PER_CORE:(c + 1) * ROWS_PER_CORE],
            "g_aug": g_aug,
            "prior_zdT": prior_zdT,
            "wfc_pad": wfc_pad,
            "ones_b": np.ones((1, R), ml_dtypes.bfloat16),
        })
    res = run_bass_kernel_spmd(nc, in_maps, list(range(NCORES)), trace=_trace)
    # per-core out is [IMG, C, R]; un-transpose to [IMG*R, C]
    out = np.concatenate(
        [np.asarray(r["out"]).transpose(0, 2, 1).reshape(-1, C)
         for r in res.results], axis=0).astype(np.float32)
    if _trace:
        return out, res
    return out


if __name__ == "__main__":
    rng = np.random.default_rng(0)
    inputs = {
        "x": rng.standard_normal((B * R, C), dtype=np.float32),
        "Wq": rng.standard_normal((C, DK), dtype=np.float32) / 9.0,
        "bq": np.zeros(DK, np.float32),
        "Wk": rng.standard_normal((C, DK), dtype=np.float32) / 9.0,
        "bk": np.zeros(DK, np.float32),
        "Wfc": rng.standard_normal((C, C), dtype=np.float32) / 9.0,
        "bfc": np.zeros(C, np.float32),
        "prior_rel": rng.random((C, C), dtype=np.float32),
    }
    out = kernel(**inputs)
    print("out", out.shape, out.dtype, float(out.mean()))
